# revision 11
# baseline (speedup 1.0000x reference)
"""Causal dot-product attention (B=4, S=2048, D=1024) on 8 TRN2 NeuronCores.

Sharding: batch x query-tile-class. Core c handles batch c//2; the 16
query row-tiles (128 rows each) of a batch are split between its two
cores so that both get the same padded causal-extent sequence (in
512-key blocks, descending) [4,4,3,3,2,2,1,1] -> one SPMD program for
all 8 cores. Projection weights are replicated.

Numerics: projections and QK^T run in fp32r (11-bit-mantissa fp32, full
PE speed); softmax stats in fp32; probs and V in bf16 for the SV matmul.
The fp32r input-rounding error in the q/k projections is dominated by a
rank-1 common mode (rowsum(x - round(x)) x colmean(W)); it is cancelled
exactly: rowsum(x) comes from accumulate-DMAs + a GpSimd partition
all-reduce, rowsum(round(x)) from a DVE reduction of the rounded input,
colmean(W) from accumulate-DMAs, and the rank-1 update is added to
k1/q1 via cheap K=1 matmuls. The causal mask is applied from the real
mask input via a fused (mask*2^19 + logits) op before max-subtraction.
"""
import numpy as np
import concourse.bass as bass
import concourse.bass_isa as bass_isa
import concourse.mybir as mybir
from concourse import bacc
from concourse.tile import TileContext
from concourse.bass_utils import run_bass_kernel_spmd
from concourse.masks import make_identity

f32 = mybir.dt.float32
f32r = mybir.dt.float32r
bf16 = mybir.dt.bfloat16
u8 = mybir.dt.uint8
AF = mybir.ActivationFunctionType
ALU = mybir.AluOpType
RED = bass_isa.ReduceOp

B, S, D = 4, 2048, 1024
SH = 1024                 # query rows per core
NSLOT = 8                 # 128-row query tiles per core
NBLK = [4, 4, 3, 3, 2, 2, 1, 1]   # padded extent per slot, in 512-key blocks
TILES = [[12, 13, 8, 9, 4, 5, 0, 1], [14, 15, 10, 11, 6, 7, 2, 3]]
MOFF = float(2 ** 19)     # mask offset; >> max |logit| (~1.4e5)
SCALE = 1.0 / 32.0        # 1/sqrt(D)


def build(correct=True):
    nc = bacc.Bacc()
    qT = nc.dram_tensor("qT", [D, SH], f32, kind="ExternalInput")
    kT = nc.dram_tensor("kT", [D, S], f32, kind="ExternalInput")
    vT = nc.dram_tensor("vT", [D, S], f32, kind="ExternalInput")
    Wq = nc.dram_tensor("Wq", [D, D], f32, kind="ExternalInput")
    Wk = nc.dram_tensor("Wk", [D, D], f32, kind="ExternalInput")
    Wv = nc.dram_tensor("Wv", [D, D], f32, kind="ExternalInput")
    Mu = nc.dram_tensor("Mu", [SH, S], u8, kind="ExternalInput")
    O = nc.dram_tensor("O", [SH, D], f32, kind="ExternalOutput")

    qT3 = qT.rearrange("(c p) n -> p c n", p=128)   # [128, 8, 1024]
    kT3 = kT.rearrange("(c p) n -> p c n", p=128)   # [128, 8, 2048]
    vT3 = vT.rearrange("(c p) n -> p c n", p=128)
    Wq3 = Wq.rearrange("(c p) n -> p c n", p=128)
    Wk3 = Wk.rearrange("(c p) n -> p c n", p=128)
    Wv3 = Wv.rearrange("(c p) n -> p c n", p=128)

    def load_whalf(pool, W3, half, tag):
        """One [128, 8, 512] f32r half-of-W tile, DMA'd in two quarters."""
        w = pool.tile([128, 8, 512], f32r, tag=tag)
        base = half * 512
        nc.gpsimd.dma_start(out=w[:, :, 0:256], in_=W3[:, :, base:base + 256])
        nc.gpsimd.dma_start(out=w[:, :, 256:512], in_=W3[:, :, base + 256:base + 512])
        return w

    def colmean_chain(corr_pool, scr_pool, W3, ctag):
        """c[1, D] (f32r) = colmean(W) via accumulate-DMAs + partition all-reduce."""
        c = corr_pool.tile([1, D], f32r, tag=ctag)
        for h in range(2):
            sl = slice(h * 512, (h + 1) * 512)
            wacc = scr_pool.tile([128, 512], f32, tag="scr")
            nc.gpsimd.dma_start(out=wacc, in_=W3[:, 0, sl])
            for cch in range(1, 8):
                nc.gpsimd.dma_start(out=wacc, in_=W3[:, cch, sl], accum_op=ALU.add)
            cred = scr_pool.tile([128, 512], f32, tag="scr")
            nc.gpsimd.partition_all_reduce(cred[:], wacc[:], channels=128, reduce_op=RED.add)
            nc.vector.tensor_scalar_mul(c[0:1, sl], cred[0:1, :], 1.0 / 1024.0)
        return c

    def rowsum_exact(corr_pool, scr_pool, X3, nsb, rstag):
        """rs[1, 512*nsb] (f32) = exact per-column sums of X (contraction dim)."""
        rs = corr_pool.tile([1, 512 * nsb], f32, tag=rstag)
        for sb in range(nsb):
            xacc = scr_pool.tile([128, 512], f32, tag="scr")
            sl = slice(sb * 512, (sb + 1) * 512)
            nc.gpsimd.dma_start(out=xacc, in_=X3[:, 0, sl])
            for cch in range(1, 8):
                nc.gpsimd.dma_start(out=xacc, in_=X3[:, cch, sl], accum_op=ALU.add)
            rso = scr_pool.tile([128, 512], f32, tag="scr")
            nc.gpsimd.partition_all_reduce(rso[:], xacc[:], channels=128, reduce_op=RED.add)
            nc.vector.tensor_copy(rs[0:1, sl], rso[0:1, :])
        return rs

    def rsr_chunk(scr_pool, it, drow, rs, sb):
        """drow[1, sb-slice] (f32r) = rs - rowsum(rounded chunk)."""
        tmp = scr_pool.tile([128, 512], f32, tag="scr")
        nc.vector.tensor_tensor(out=tmp[:], in0=it[:, 0, :], in1=it[:, 1, :], op=ALU.add)
        for cch in range(2, 8):
            nc.vector.tensor_tensor(out=tmp[:], in0=tmp[:], in1=it[:, cch, :], op=ALU.add)
        rro = scr_pool.tile([128, 512], f32, tag="scr")
        nc.gpsimd.partition_all_reduce(rro[:], tmp[:], channels=128, reduce_op=RED.add)
        sl = slice(sb * 512, (sb + 1) * 512)
        nc.vector.tensor_tensor(out=drow[0:1, sl], in0=rs[0:1, sl], in1=rro[0:1, :],
                                op=ALU.subtract)

    with TileContext(nc) as tc:
        with tc.tile_pool(name="pers", bufs=1) as pers:
            k1T = pers.tile([128, 8, S], f32r, tag="k1T")      # 64 KB/part
            v1 = pers.tile([128, 16, D], bf16, tag="v1")       # 32 KB/part

            # left stack: inp | corrK | wk0 wk1
            inp = tc.alloc_tile_pool(name="inp", bufs=2, side="left")
            scrK = tc.alloc_tile_pool(name="scrK", bufs=3, side="left") if correct else None
            corrK = tc.alloc_tile_pool(name="corrK", bufs=1, side="left") if correct else None
            wk0_p = tc.alloc_tile_pool(name="wk0", bufs=1, side="left")
            wk1_p = tc.alloc_tile_pool(name="wk1", bufs=1, side="left")
            # right stack: wv0 (preloaded during K)
            wv0_p = tc.alloc_tile_pool(name="wv0", bufs=1, side="right")

            pps = tc.alloc_tile_pool(name="pps", bufs=4, space="PSUM")
            cps = tc.alloc_tile_pool(name="cps", bufs=2, space="PSUM") if correct else None

            # =============== phase K: k1T = Wk^T kT (+ correction) ===============
            wk = [load_whalf(wk0_p, Wk3, 0, "wk0"), load_whalf(wk1_p, Wk3, 1, "wk1")]
            if correct:
                c_k = colmean_chain(corrK, scrK, Wk3, "c_k")
                rs_k = rowsum_exact(corrK, scrK, kT3, 4, "rs_k")
                drow_k = corrK.tile([1, S], f32r, tag="drow_k")
            wv = [None, None]
            for sb in range(4):
                it = inp.tile([128, 8, 512], f32r, tag="inT")
                nc.gpsimd.dma_start(out=it, in_=kT3[:, :, sb * 512:(sb + 1) * 512])
                if correct:
                    rsr_chunk(scrK, it, drow_k, rs_k, sb)
                for dout in range(8):
                    ps = pps.tile([128, 512], f32, tag="pp")
                    for din in range(8):
                        nc.tensor.matmul(
                            ps[:], wk[dout // 4][:, din, (dout % 4) * 128:(dout % 4 + 1) * 128],
                            it[:, din, :], start=(din == 0), stop=(din == 7))
                    nc.vector.tensor_copy(
                        k1T[:, dout, sb * 512:(sb + 1) * 512], ps[:])
                if sb == 1:
                    wv[0] = load_whalf(wv0_p, Wv3, 0, "wv0")
            # correction sweep: k1T[dout, s-blk] += c_k[dout-cols] (x) drow_k[s-blk]
            if correct:
                for sb in range(4):
                    for dout in range(8):
                        pc = cps.tile([128, 512], f32, tag="pc")
                        nc.tensor.matmul(
                            pc[:], c_k[0:1, dout * 128:(dout + 1) * 128],
                            drow_k[0:1, sb * 512:(sb + 1) * 512], start=True, stop=True)
                        sl = slice(sb * 512, (sb + 1) * 512)
                        nc.vector.tensor_tensor(
                            out=k1T[:, dout, sl], in0=k1T[:, dout, sl], in1=pc[:],
                            op=ALU.add)
            wk1_p.release()
            wk0_p.release()
            if correct:
                corrK.release()
                scrK.release()
                scrQ = tc.alloc_tile_pool(name="scrQ", bufs=2, side="left")
                corrQ = tc.alloc_tile_pool(name="corrQ", bufs=1, side="left")

            # =============== phase V: v1 = vT^T Wv (no correction) ===============
            wv1_p = tc.alloc_tile_pool(name="wv1", bufs=1, side="right")
            wv[1] = load_whalf(wv1_p, Wv3, 1, "wv1")
            if correct:
                c_q = colmean_chain(corrQ, scrQ, Wq3, "c_q")
                rs_q = rowsum_exact(corrQ, scrQ, qT3, 2, "rs_q")
                drow_q = corrQ.tile([1, SH], f32r, tag="drow_q")
            wq = [None, None]
            wq_pool = tc.alloc_tile_pool(name="wq", bufs=1, side="left")
            for sb in range(4):
                it = inp.tile([128, 8, 512], f32r, tag="inT")
                nc.gpsimd.dma_start(out=it, in_=vT3[:, :, sb * 512:(sb + 1) * 512])
                for kc in range(4):
                    for dv in range(2):
                        ps = pps.tile([128, 512], f32, tag="pp")
                        for din in range(8):
                            nc.tensor.matmul(
                                ps[:], it[:, din, kc * 128:(kc + 1) * 128],
                                wv[dv][:, din, :], start=(din == 0), stop=(din == 7))
                        nc.vector.tensor_copy(
                            v1[:, sb * 4 + kc, dv * 512:(dv + 1) * 512], ps[:])
                if sb == 1:
                    wq[0] = load_whalf(wq_pool, Wq3, 0, "wq")
            wv1_p.release()
            wv0_p.release()

            # =============== phase Q: q1T = (1/32) Wq^T qT (+ correction) ===============
            # W halves streamed through one slot; qT chunks re-read per half
            q1_pool = tc.alloc_tile_pool(name="q1p", bufs=1, side="right")
            q1T = q1_pool.tile([128, 8, SH], f32r, tag="q1T")  # 32 KB/part
            for wh in range(2):
                w = wq[0] if wh == 0 else load_whalf(wq_pool, Wq3, 1, "wq")
                for sb in range(2):
                    it = inp.tile([128, 8, 512], f32r, tag="inT")
                    nc.gpsimd.dma_start(out=it, in_=qT3[:, :, sb * 512:(sb + 1) * 512])
                    if correct and wh == 0:
                        rsr_chunk(scrQ, it, drow_q, rs_q, sb)
                    for d4 in range(4):
                        dout = wh * 4 + d4
                        ps = pps.tile([128, 512], f32, tag="pp")
                        for din in range(8):
                            nc.tensor.matmul(
                                ps[:], w[:, din, d4 * 128:(d4 + 1) * 128],
                                it[:, din, :], start=(din == 0), stop=(din == 7))
                        nc.vector.tensor_scalar_mul(
                            q1T[:, dout, sb * 512:(sb + 1) * 512], ps[:], SCALE)
            # correction sweep (scaled by 1/32), slot-0 columns first
            if correct:
                for sb in range(2):
                    for dout in range(8):
                        pc = cps.tile([128, 512], f32, tag="pc")
                        nc.tensor.matmul(
                            pc[:], c_q[0:1, dout * 128:(dout + 1) * 128],
                            drow_q[0:1, sb * 512:(sb + 1) * 512], start=True, stop=True)
                        sl = slice(sb * 512, (sb + 1) * 512)
                        nc.vector.scalar_tensor_tensor(
                            q1T[:, dout, sl], pc[:], SCALE, q1T[:, dout, sl],
                            op0=ALU.mult, op1=ALU.add)
            wq_pool.release()
            if correct:
                corrQ.release()
                scrQ.release()
            inp.release()
            if correct:
                cps.release()
            pps.release()

            # ---- attention, one 128-row query tile per slot ----
            with (
                tc.tile_pool(name="work", bufs=2) as work,
                tc.tile_pool(name="small", bufs=2) as small,
                tc.tile_pool(name="qkps", bufs=3, space="PSUM") as qkps,
                tc.tile_pool(name="tpps", bufs=2, space="PSUM") as tpps,
                tc.tile_pool(name="svps", bufs=3, space="PSUM") as svps,
            ):
                ident = work.tile([128, 128], bf16, tag="ident")
                make_identity(nc, ident[:])
                for s in range(NSLOT):
                    nblk = NBLK[s]
                    E = 4 * nblk              # extent in 128-key chunks
                    L = 512 * nblk            # extent in keys
                    mu = work.tile([128, 4, 512], u8, tag="mu")
                    nc.sync.dma_start(out=mu[:, :nblk, :], in_=Mu[s * 128:(s + 1) * 128, :L])
                    logits = work.tile([128, 4, 512], f32, tag="lg")
                    maxs = small.tile([128, 4], f32, tag="maxs")
                    for j4 in range(nblk):
                        qk = qkps.tile([128, 512], f32, tag="qk")
                        for din in range(8):
                            nc.tensor.matmul(
                                qk[:],
                                q1T[:, din, s * 128:(s + 1) * 128],
                                k1T[:, din, j4 * 512:(j4 + 1) * 512],
                                start=(din == 0), stop=(din == 7))
                        # logits = mask*2^19 + qk  (allowed ~2^19, masked small)
                        nc.vector.scalar_tensor_tensor(
                            logits[:, j4, :], mu[:, j4, :], MOFF, qk[:],
                            op0=ALU.mult, op1=ALU.add)
                        nc.vector.tensor_reduce(
                            maxs[:, j4:j4 + 1], logits[:, j4, :],
                            axis=mybir.AxisListType.X, op=ALU.max)
                    negmax = small.tile([128, 1], f32, tag="negmax")
                    nc.vector.tensor_reduce(
                        negmax[:], maxs[:, :nblk], axis=mybir.AxisListType.X,
                        op=ALU.max, negate=True)
                    # exp + per-block row sums
                    probs = work.tile([128, 4, 512], bf16, tag="probs")
                    sums = small.tile([128, 4], f32, tag="sums")
                    for j4 in range(nblk):
                        nc.scalar.activation(
                            probs[:, j4, :], logits[:, j4, :], AF.Exp,
                            bias=negmax[:, 0:1], scale=1.0,
                            accum_out=sums[:, j4:j4 + 1])
                    total = small.tile([128, 1], f32, tag="total")
                    nc.vector.tensor_reduce(
                        total[:], sums[:, :nblk], axis=mybir.AxisListType.X, op=ALU.add)
                    recip = small.tile([128, 1], f32, tag="recip")
                    nc.vector.reciprocal(recip[:], total[:])
                    # transpose probs 128x128 blocks (PE)
                    pT = work.tile([128, 16, 128], bf16, tag="pT")
                    p2 = probs[:].rearrange("p a b -> p (a b)")
                    for j in range(E):
                        tp = tpps.tile([128, 128], bf16, tag="tp")
                        nc.tensor.transpose(tp[:], p2[:, j * 128:(j + 1) * 128], ident[:])
                        nc.vector.tensor_copy(pT[:, j, :], tp[:])
                    # SV: out[q, dv] = sum_j pT[j].T @ v1[j, dv]
                    ot = work.tile([128, D], f32, tag="ot")
                    for dv in range(2):
                        sv = svps.tile([128, 512], f32, tag="sv")
                        for j in range(E):
                            nc.tensor.matmul(
                                sv[:], pT[:, j, :], v1[:, j, dv * 512:(dv + 1) * 512],
                                start=(j == 0), stop=(j == E - 1))
                        # normalize by 1/rowsum during evacuation
                        nc.scalar.activation(
                            ot[:, dv * 512:(dv + 1) * 512], sv[:], AF.Copy,
                            bias=0.0, scale=recip[:, 0:1])
                    nc.sync.dma_start(out=O[s * 128:(s + 1) * 128, :], in_=ot[:])
            q1_pool.release()
    nc.finalize()
    return nc


_NC_CACHE = []


def kernel(q, k, v, mask, W_q, W_k, W_v):
    q = np.asarray(q, dtype=np.float32)
    k = np.asarray(k, dtype=np.float32)
    v = np.asarray(v, dtype=np.float32)
    W_q = np.asarray(W_q, dtype=np.float32)
    W_k = np.asarray(W_k, dtype=np.float32)
    W_v = np.asarray(W_v, dtype=np.float32)
    mask_u8 = np.asarray(mask).astype(np.uint8)

    if not _NC_CACHE:
        _NC_CACHE.append(build())
    nc = _NC_CACHE[0]

    row_sets = []
    in_maps = []
    for c in range(8):
        b, cls = c // 2, c % 2
        rows = np.concatenate([np.arange(128 * t, 128 * (t + 1)) for t in TILES[cls]])
        row_sets.append((b, rows))
        in_maps.append({
            "qT": np.ascontiguousarray(q[b][rows, :].T),
            "kT": np.ascontiguousarray(k[b].T),
            "vT": np.ascontiguousarray(v[b].T),
            "Wq": W_q, "Wk": W_k, "Wv": W_v,
            "Mu": np.ascontiguousarray(mask_u8[b][rows, :]),
        })

    res = run_bass_kernel_spmd(nc, in_maps, core_ids=list(range(8)))

    out = np.empty((B, S, D), dtype=np.float32)
    for c in range(8):
        b, rows = row_sets[c]
        out[b][rows, :] = res.results[c]["O"]
    return out


# revision 13
# speedup vs baseline: 1.7695x; 1.7695x over previous
"""Causal dot-product attention (B=4, S=2048, D=1024) on 8 TRN2 NeuronCores.

Sharding: batch x query-tile-class. Core c handles batch c//2; the 16
query row-tiles (128 rows each) of a batch are split between its two
cores so that both get the same padded causal-extent sequence (in
512-key blocks, descending) [4,4,3,3,2,2,1,1] -> one SPMD program for
all 8 cores. Projection weights are replicated.

Numerics: projections and QK^T run in fp32r (11-bit-mantissa fp32, full
PE speed); softmax stats in fp32; probs and V in bf16 for the SV matmul.
The fp32r input-rounding error in the q/k projections is dominated by a
rank-1 common mode (rowsum(x - round(x)) x colmean(W)); it is cancelled
exactly: rowsum(x) comes from accumulate-DMAs + a GpSimd partition
all-reduce, rowsum(round(x)) from a DVE reduction of the rounded input,
colmean(W) from accumulate-DMAs, and the rank-1 update is added to
k1/q1 via cheap K=1 matmuls. The causal mask is applied from the real
mask input via a fused (mask*2^19 + logits) op before max-subtraction.
"""
import numpy as np
import concourse.bass as bass
import concourse.bass_isa as bass_isa
import concourse.mybir as mybir
from concourse import bacc
from concourse.tile import TileContext
from concourse.bass_utils import run_bass_kernel_spmd
from concourse.masks import make_identity

f32 = mybir.dt.float32
f32r = mybir.dt.float32r
bf16 = mybir.dt.bfloat16
u8 = mybir.dt.uint8
AF = mybir.ActivationFunctionType
ALU = mybir.AluOpType
RED = bass_isa.ReduceOp

B, S, D = 4, 2048, 1024
SH = 1024                 # query rows per core
NSLOT = 8                 # 128-row query tiles per core
NBLK = [4, 4, 3, 3, 2, 2, 1, 1]   # padded extent per slot, in 512-key blocks
TILES = [[12, 13, 8, 9, 4, 5, 0, 1], [14, 15, 10, 11, 6, 7, 2, 3]]
MOFF = float(2 ** 19)     # mask offset; >> max |logit| (~1.4e5)
SCALE = 1.0 / 32.0        # 1/sqrt(D)


def build(correct=True):
    nc = bacc.Bacc()
    qT = nc.dram_tensor("qT", [D, SH], f32, kind="ExternalInput")
    kT = nc.dram_tensor("kT", [D, S], f32, kind="ExternalInput")
    vT = nc.dram_tensor("vT", [D, S], f32, kind="ExternalInput")
    Wq = nc.dram_tensor("Wq", [D, D], f32, kind="ExternalInput")
    Wk = nc.dram_tensor("Wk", [D, D], f32, kind="ExternalInput")
    Wv = nc.dram_tensor("Wv", [D, D], f32, kind="ExternalInput")
    Mu = nc.dram_tensor("Mu", [SH, S], u8, kind="ExternalInput")
    O = nc.dram_tensor("O", [SH, D], f32, kind="ExternalOutput")

    qT3 = qT.rearrange("(c p) n -> p c n", p=128)   # [128, 8, 1024]
    kT3 = kT.rearrange("(c p) n -> p c n", p=128)   # [128, 8, 2048]
    vT3 = vT.rearrange("(c p) n -> p c n", p=128)
    Wq3 = Wq.rearrange("(c p) n -> p c n", p=128)
    Wk3 = Wk.rearrange("(c p) n -> p c n", p=128)
    Wv3 = Wv.rearrange("(c p) n -> p c n", p=128)

    def load_whalf(pool, W3, half, tag):
        """One [128, 8, 512] f32r half-of-W tile, DMA'd in two quarters."""
        w = pool.tile([128, 8, 512], f32r, tag=tag)
        base = half * 512
        nc.gpsimd.dma_start(out=w[:, :, 0:256], in_=W3[:, :, base:base + 256])
        nc.gpsimd.dma_start(out=w[:, :, 256:512], in_=W3[:, :, base + 256:base + 512])
        return w

    def colmean_pe(corr_pool, cps, ones_r, w_half, c, h):
        """c[1, h-half] (f32r) = colmean of resident f32r W half via ones-matmuls."""
        pcs = cps.tile([1, 512], f32, tag="pcs")
        for din in range(8):
            nc.tensor.matmul(pcs[:], ones_r[:, 0:1], w_half[:, din, :],
                             start=(din == 0), stop=(din == 7))
        nc.vector.tensor_scalar_mul(c[0:1, h * 512:(h + 1) * 512], pcs[0:1, :],
                                    1.0 / 1024.0)

    def d_chunk_half(lo_pool, cps, ones_r, stg, it, pd, hh):
        """pd[1, hh-256-slice] = rowsum(x - round(x)) for a half-chunk (stg is [128,8,256])."""
        for din in range(8):
            lo = lo_pool.tile([128, 256], f32r, tag="lo")
            nc.vector.tensor_tensor(out=lo[:], in0=stg[:, din, :],
                                    in1=it[:, din, hh * 256:(hh + 1) * 256],
                                    op=ALU.subtract)
            nc.tensor.matmul(pd[:, hh * 256:(hh + 1) * 256], ones_r[:, 0:1], lo[:],
                             start=(din == 0), stop=(din == 7))

    with TileContext(nc) as tc:
        with tc.tile_pool(name="pers", bufs=1) as pers:
            k1T = pers.tile([128, 8, S], f32r, tag="k1T")      # 64 KB/part
            v1 = pers.tile([128, 16, D], bf16, tag="v1")       # 32 KB/part

            # left stack: inp | corrK | wk0 wk1
            inp = tc.alloc_tile_pool(name="inp", bufs=2, side="left")
            loK = tc.alloc_tile_pool(name="loK", bufs=3, side="left") if correct else None
            corrK = tc.alloc_tile_pool(name="corrK", bufs=1, side="left") if correct else None
            wk0_p = tc.alloc_tile_pool(name="wk0", bufs=1, side="left")
            wk1_p = tc.alloc_tile_pool(name="wk1", bufs=1, side="left")
            # right stack: wv0 (preloaded during K)
            wv0_p = tc.alloc_tile_pool(name="wv0", bufs=1, side="right")

            pps = tc.alloc_tile_pool(name="pps", bufs=4, space="PSUM")
            cps = tc.alloc_tile_pool(name="cps", bufs=2, space="PSUM") if correct else None

            # =============== phase K: k1T = Wk^T kT (+ correction) ===============
            wk = [load_whalf(wk0_p, Wk3, 0, "wk0"), load_whalf(wk1_p, Wk3, 1, "wk1")]
            if correct:
                onef = corrK.tile([128, 1], f32, tag="onef")
                nc.vector.memset(onef[:], 1.0)
                ones_r = corrK.tile([128, 1], f32r, tag="oner")
                nc.vector.tensor_copy(ones_r[:], onef[:])
                c_k = corrK.tile([1, D], f32r, tag="c_k")
                for h in range(2):
                    colmean_pe(corrK, cps, ones_r, wk[h], c_k, h)
                drow_k = corrK.tile([1, S], f32r, tag="drow_k")
            wv = [None, None]
            for sb in range(4):
                it = inp.tile([128, 8, 512], f32r, tag="inT")
                nc.gpsimd.dma_start(out=it, in_=kT3[:, :, sb * 512:(sb + 1) * 512])
                for dout in range(8):
                    ps = pps.tile([128, 512], f32, tag="pp")
                    for din in range(8):
                        nc.tensor.matmul(
                            ps[:], wk[dout // 4][:, din, (dout % 4) * 128:(dout % 4 + 1) * 128],
                            it[:, din, :], start=(din == 0), stop=(din == 7))
                    nc.vector.tensor_copy(
                        k1T[:, dout, sb * 512:(sb + 1) * 512], ps[:])
                if correct:
                    pd = cps.tile([1, 512], f32, tag="pcs")
                    for hh in range(2):
                        stg = corrK.tile([128, 8, 256], f32, tag="stg")
                        base = sb * 512 + hh * 256
                        nc.sync.dma_start(out=stg, in_=kT3[:, :, base:base + 256])
                        d_chunk_half(loK, cps, ones_r, stg, it, pd, hh)
                    nc.vector.tensor_copy(drow_k[0:1, sb * 512:(sb + 1) * 512], pd[0:1, :])
                if sb == 1:
                    wv[0] = load_whalf(wv0_p, Wv3, 0, "wv0")
            # correction sweep: k1T[dout, s-blk] += c_k[dout-cols] (x) drow_k[s-blk]
            if correct:
                for sb in range(4):
                    for dout in range(8):
                        pc = cps.tile([128, 512], f32, tag="pc")
                        nc.tensor.matmul(
                            pc[:], c_k[0:1, dout * 128:(dout + 1) * 128],
                            drow_k[0:1, sb * 512:(sb + 1) * 512], start=True, stop=True)
                        sl = slice(sb * 512, (sb + 1) * 512)
                        nc.vector.tensor_tensor(
                            out=k1T[:, dout, sl], in0=k1T[:, dout, sl], in1=pc[:],
                            op=ALU.add)
            wk1_p.release()
            wk0_p.release()
            if correct:
                corrK.release()
                loK.release()
                loQ = tc.alloc_tile_pool(name="loQ", bufs=3, side="left")
                corrQ = tc.alloc_tile_pool(name="corrQ", bufs=1, side="left")

            # =============== phase V: v1 = vT^T Wv (no correction) ===============
            wv1_p = tc.alloc_tile_pool(name="wv1", bufs=1, side="right")
            wv[1] = load_whalf(wv1_p, Wv3, 1, "wv1")
            if correct:
                onefq = corrQ.tile([128, 1], f32, tag="onef")
                nc.vector.memset(onefq[:], 1.0)
                ones_rq = corrQ.tile([128, 1], f32r, tag="oner")
                nc.vector.tensor_copy(ones_rq[:], onefq[:])
                c_q = corrQ.tile([1, D], f32r, tag="c_q")
                drow_q = corrQ.tile([1, SH], f32r, tag="drow_q")
            wq = [None, None]
            wq_pool = tc.alloc_tile_pool(name="wq", bufs=1, side="left")
            for sb in range(4):
                it = inp.tile([128, 8, 512], f32r, tag="inT")
                nc.gpsimd.dma_start(out=it, in_=vT3[:, :, sb * 512:(sb + 1) * 512])
                for kc in range(4):
                    for dv in range(2):
                        ps = pps.tile([128, 512], f32, tag="pp")
                        for din in range(8):
                            nc.tensor.matmul(
                                ps[:], it[:, din, kc * 128:(kc + 1) * 128],
                                wv[dv][:, din, :], start=(din == 0), stop=(din == 7))
                        nc.vector.tensor_copy(
                            v1[:, sb * 4 + kc, dv * 512:(dv + 1) * 512], ps[:])
                if sb == 1:
                    wq[0] = load_whalf(wq_pool, Wq3, 0, "wq")
            wv1_p.release()
            wv0_p.release()

            # =============== phase Q: q1T = (1/32) Wq^T qT (+ correction) ===============
            # W halves streamed through one slot; qT chunks re-read per half
            q1_pool = tc.alloc_tile_pool(name="q1p", bufs=1, side="right")
            q1T = q1_pool.tile([128, 8, SH], f32r, tag="q1T")  # 32 KB/part
            for wh in range(2):
                w = wq[0] if wh == 0 else load_whalf(wq_pool, Wq3, 1, "wq")
                if correct:
                    colmean_pe(corrQ, cps, ones_rq, w, c_q, wh)
                for sb in range(2):
                    it = inp.tile([128, 8, 512], f32r, tag="inT")
                    nc.gpsimd.dma_start(out=it, in_=qT3[:, :, sb * 512:(sb + 1) * 512])
                    for d4 in range(4):
                        dout = wh * 4 + d4
                        ps = pps.tile([128, 512], f32, tag="pp")
                        for din in range(8):
                            nc.tensor.matmul(
                                ps[:], w[:, din, d4 * 128:(d4 + 1) * 128],
                                it[:, din, :], start=(din == 0), stop=(din == 7))
                        nc.vector.tensor_scalar_mul(
                            q1T[:, dout, sb * 512:(sb + 1) * 512], ps[:], SCALE)
                    if correct and wh == 0:
                        pd = cps.tile([1, 512], f32, tag="pcs")
                        for hh in range(2):
                            stg = corrQ.tile([128, 8, 256], f32, tag="stg")
                            base = sb * 512 + hh * 256
                            nc.sync.dma_start(out=stg, in_=qT3[:, :, base:base + 256])
                            d_chunk_half(loQ, cps, ones_rq, stg, it, pd, hh)
                        nc.vector.tensor_copy(drow_q[0:1, sb * 512:(sb + 1) * 512], pd[0:1, :])
            # correction sweep (scaled by 1/32), slot-0 columns first
            if correct:
                for sb in range(2):
                    for dout in range(8):
                        pc = cps.tile([128, 512], f32, tag="pc")
                        nc.tensor.matmul(
                            pc[:], c_q[0:1, dout * 128:(dout + 1) * 128],
                            drow_q[0:1, sb * 512:(sb + 1) * 512], start=True, stop=True)
                        sl = slice(sb * 512, (sb + 1) * 512)
                        nc.vector.scalar_tensor_tensor(
                            q1T[:, dout, sl], pc[:], SCALE, q1T[:, dout, sl],
                            op0=ALU.mult, op1=ALU.add)
            wq_pool.release()
            if correct:
                corrQ.release()
                loQ.release()
            inp.release()
            if correct:
                cps.release()
            pps.release()

            # ---- attention, one 128-row query tile per slot ----
            with (
                tc.tile_pool(name="work", bufs=2) as work,
                tc.tile_pool(name="small", bufs=2) as small,
                tc.tile_pool(name="qkps", bufs=3, space="PSUM") as qkps,
                tc.tile_pool(name="tpps", bufs=2, space="PSUM") as tpps,
                tc.tile_pool(name="svps", bufs=3, space="PSUM") as svps,
            ):
                ident = work.tile([128, 128], bf16, tag="ident")
                make_identity(nc, ident[:])
                for s in range(NSLOT):
                    nblk = NBLK[s]
                    E = 4 * nblk              # extent in 128-key chunks
                    L = 512 * nblk            # extent in keys
                    mu = work.tile([128, 4, 512], u8, tag="mu")
                    nc.sync.dma_start(out=mu[:, :nblk, :], in_=Mu[s * 128:(s + 1) * 128, :L])
                    logits = work.tile([128, 4, 512], f32, tag="lg")
                    maxs = small.tile([128, 4], f32, tag="maxs")
                    for j4 in range(nblk):
                        qk = qkps.tile([128, 512], f32, tag="qk")
                        for din in range(8):
                            nc.tensor.matmul(
                                qk[:],
                                q1T[:, din, s * 128:(s + 1) * 128],
                                k1T[:, din, j4 * 512:(j4 + 1) * 512],
                                start=(din == 0), stop=(din == 7))
                        # logits = mask*2^19 + qk  (allowed ~2^19, masked small)
                        nc.vector.scalar_tensor_tensor(
                            logits[:, j4, :], mu[:, j4, :], MOFF, qk[:],
                            op0=ALU.mult, op1=ALU.add)
                        nc.vector.tensor_reduce(
                            maxs[:, j4:j4 + 1], logits[:, j4, :],
                            axis=mybir.AxisListType.X, op=ALU.max)
                    negmax = small.tile([128, 1], f32, tag="negmax")
                    nc.vector.tensor_reduce(
                        negmax[:], maxs[:, :nblk], axis=mybir.AxisListType.X,
                        op=ALU.max, negate=True)
                    # exp + per-block row sums
                    probs = work.tile([128, 4, 512], bf16, tag="probs")
                    sums = small.tile([128, 4], f32, tag="sums")
                    for j4 in range(nblk):
                        nc.scalar.activation(
                            probs[:, j4, :], logits[:, j4, :], AF.Exp,
                            bias=negmax[:, 0:1], scale=1.0,
                            accum_out=sums[:, j4:j4 + 1])
                    total = small.tile([128, 1], f32, tag="total")
                    nc.vector.tensor_reduce(
                        total[:], sums[:, :nblk], axis=mybir.AxisListType.X, op=ALU.add)
                    recip = small.tile([128, 1], f32, tag="recip")
                    nc.vector.reciprocal(recip[:], total[:])
                    # transpose probs 128x128 blocks (PE)
                    pT = work.tile([128, 16, 128], bf16, tag="pT")
                    p2 = probs[:].rearrange("p a b -> p (a b)")
                    for j in range(E):
                        tp = tpps.tile([128, 128], bf16, tag="tp")
                        nc.tensor.transpose(tp[:], p2[:, j * 128:(j + 1) * 128], ident[:])
                        nc.vector.tensor_copy(pT[:, j, :], tp[:])
                    # SV: out[q, dv] = sum_j pT[j].T @ v1[j, dv]
                    ot = work.tile([128, D], f32, tag="ot")
                    for dv in range(2):
                        sv = svps.tile([128, 512], f32, tag="sv")
                        for j in range(E):
                            nc.tensor.matmul(
                                sv[:], pT[:, j, :], v1[:, j, dv * 512:(dv + 1) * 512],
                                start=(j == 0), stop=(j == E - 1))
                        # normalize by 1/rowsum during evacuation
                        nc.scalar.activation(
                            ot[:, dv * 512:(dv + 1) * 512], sv[:], AF.Copy,
                            bias=0.0, scale=recip[:, 0:1])
                    nc.sync.dma_start(out=O[s * 128:(s + 1) * 128, :], in_=ot[:])
            q1_pool.release()
    nc.finalize()
    return nc


_NC_CACHE = []


def kernel(q, k, v, mask, W_q, W_k, W_v):
    q = np.asarray(q, dtype=np.float32)
    k = np.asarray(k, dtype=np.float32)
    v = np.asarray(v, dtype=np.float32)
    W_q = np.asarray(W_q, dtype=np.float32)
    W_k = np.asarray(W_k, dtype=np.float32)
    W_v = np.asarray(W_v, dtype=np.float32)
    mask_u8 = np.asarray(mask).astype(np.uint8)

    if not _NC_CACHE:
        _NC_CACHE.append(build())
    nc = _NC_CACHE[0]

    row_sets = []
    in_maps = []
    for c in range(8):
        b, cls = c // 2, c % 2
        rows = np.concatenate([np.arange(128 * t, 128 * (t + 1)) for t in TILES[cls]])
        row_sets.append((b, rows))
        in_maps.append({
            "qT": np.ascontiguousarray(q[b][rows, :].T),
            "kT": np.ascontiguousarray(k[b].T),
            "vT": np.ascontiguousarray(v[b].T),
            "Wq": W_q, "Wk": W_k, "Wv": W_v,
            "Mu": np.ascontiguousarray(mask_u8[b][rows, :]),
        })

    res = run_bass_kernel_spmd(nc, in_maps, core_ids=list(range(8)))

    out = np.empty((B, S, D), dtype=np.float32)
    for c in range(8):
        b, rows = row_sets[c]
        out[b][rows, :] = res.results[c]["O"]
    return out
